# revision 56
# baseline (speedup 1.0000x reference)
"""Trainium2 Bass kernel for a causal single-head attention layer.

reference:
    v = inp @ Wv + bv; k = inp @ Wk + bk; q = inp @ Wq + bq      # [B,T,H]
    W = softmax(causal_mask(k @ q^T / sqrt(C)))                  # [B,T,T]
    out = W @ v                                                  # [B,T,H]

B=512, T=256, C=384, H=64. Pure data parallel over 8 NeuronCores
(64 batches each); batches are processed in QUADS (4 at a time, 16
iterations per core) with a deep software pipeline:

    iteration i issues:  A = projections(quad i)       + kt DMA
                         B = scores+exp+mask(quad i-1)
                         C = P@V+normalize+store(quad i-5)

The deep C offset gives the scores->exp->mask cross-engine chain
(~4us of serial latency) four iterations of slack, so the in-order
Tensor queue never waits on it; measured steady-state is one quad per
~3.5us with a single unbroken full-clock (HAM K=8/8) region.

Scheduling notes (all learned from NTFF traces):
  * All 8 PSUM banks are persistent per-role tiles. The qk projection
    uses two single-bank tiles evacuated separately so each bank frees
    ~1.5us into its iteration (a combined evacuation made the next
    quad's projections the critical path).
  * Input x^T tiles are prefetched 3 iterations ahead on the Sync
    queue (786KB transfers take ~2.2us on SDMA; shorter distances
    caused multi-us PE stalls and HAM down-clocks).
  * The causal mask (needed only on the two diagonal 128x128 blocks of
    the packed score layout) is applied by gpsimd affine_select after
    exp; max-subtraction is skipped (|scores/sqrt(C)| < ~3, exp cannot
    overflow; softmax is shift-invariant).
  * Softmax normalization rides a ones-column appended to V (written
    once into a persistent ring of vo tiles) so the P@V matmuls also
    emit row sums; one reciprocal + one broadcast multiply per t-block
    finish it.
  * During the pipeline drain the dead qk banks double-buffer the P@V
    output so the tail quads don't serialize on the single ou bank.
  * Output is stored as bf16 and cast to f32 on the host (within the
    accuracy budget; measured rel err ~4e-3).

Layout: scores are computed transposed (S^T[s,t], s on partitions) with
lhsT=q^T slices / rhs=k^T (k^T re-based to partition 0 by an SBUF DMA);
exp(S^T) lands in one [128, 4, 384] bf16 tile per quad which directly
provides the stationary chunks for the P@V matmuls. V is projected
directly in [s,h] layout (x^T chunks stationary), so no transposes
anywhere; the host pre-transposes inp to [NQ, C, 4, T].
"""

import numpy as np
import ml_dtypes

import concourse.bass as bass
import concourse.bacc as bacc
import concourse.mybir as mybir
import concourse.tile as tile
from concourse.bass import broadcast_tensor_aps
from concourse.bass_utils import run_bass_kernel_spmd

N_CORES = 8
B, T, C, H = 512, 256, 384, 64
NB = B // N_CORES          # batches per core
NQ = NB // 4               # batch quads per core
KC = C // 128              # contraction chunks
SCALE = C ** (-0.5)
F32 = mybir.dt.float32
BF16 = mybir.dt.bfloat16
AF = mybir.ActivationFunctionType


def _bmul(nc, out, a, b):
    a2, b2 = broadcast_tensor_aps(a, b)
    nc.vector.tensor_tensor(out, a2, b2, op=mybir.AluOpType.mult)


def _badd(nc, out, a, b):
    a2, b2 = broadcast_tensor_aps(a, b)
    nc.vector.tensor_tensor(out, a2, b2, op=mybir.AluOpType.add)


def build_nc():
    nc = bacc.Bacc("TRN2", target_bir_lowering=False, debug=False)
    x_h = nc.declare_dram_parameter("x", [NQ, C, 4, T], BF16, isOutput=False)
    wqk_h = nc.declare_dram_parameter("wqk", [C, 2 * H], BF16, isOutput=False)
    wv_h = nc.declare_dram_parameter("wv", [C, H], BF16, isOutput=False)
    bqk_h = nc.declare_dram_parameter("bqk", [128, 1], F32, isOutput=False)
    bvb_h = nc.declare_dram_parameter("bvb", [128, H], F32, isOutput=False)
    # out[q, u, p, b, h] = attention output for batch 4q+b, t = u*128+p
    out_h = nc.declare_dram_parameter("out", [NQ, 2, 128, 4, H], BF16, isOutput=True)

    with tile.TileContext(nc) as tc:
        with (
            tc.tile_pool(name="const", bufs=1) as const,
            tc.tile_pool(name="xp", bufs=5) as xp,
            tc.tile_pool(name="qkp", bufs=3) as qkp,
            tc.tile_pool(name="exp", bufs=6) as expp,
            tc.tile_pool(name="op", bufs=3) as op,
        ):
            # PE warm-up: dummy matmuls with no input dependencies so the HAM
            # clock gate ramps while the first input DMA streams.
            with tc.tile_pool(name="warm_ps", bufs=1, space="PSUM") as warm_ps:
                wsb = const.tile([128, 512], BF16, tag="wsb")
                nc.gpsimd.memset(wsb[:], 1.0)
                wps = warm_ps.tile([128, 512], F32, tag="wps")
                for _ in range(8):
                    nc.tensor.matmul(
                        wps[:], wsb[:, 0:128], wsb[:], start=True, stop=True
                    )

            ctxs = []

            def psum_pool(name, bufs=1):
                ctx = tc.tile_pool(name=name, bufs=bufs, space="PSUM")
                ctxs.append(ctx)
                return ctx.__enter__()

            # All 8 PSUM banks as persistent per-role tiles. The qk
            # projection uses two single-bank tiles (one per 2-batch group)
            # whose evacuations are split likewise, so each bank frees
            # ~1.5us into its iteration and the next quad's projections
            # never wait on a late evacuation.
            ps_ring = psum_pool("ps_ring")
            ps_qk0 = ps_ring.tile([128, 512], F32, tag="qk0", name="qk0")
            ps_qk1 = ps_ring.tile([128, 512], F32, tag="qk1", name="qk1")
            ps_v = ps_ring.tile([128, 512], F32, tag="psv", name="psv")
            ps_stA = ps_ring.tile([128, 1024], F32, tag="stA", name="stA")
            ps_stB = ps_ring.tile([128, 512], F32, tag="stB", name="stB")
            ps_ou = ps_ring.tile([128, 1024], F32, tag="psou", name="psou")

            # first quad's input DMA goes ahead of the constant loads so the
            # projection matmuls can start as soon as the warm-up drains.
            xts = {}
            xts[0] = xp.tile([128, KC, 4, T], BF16, tag="xt", name="xt0")
            nc.sync.dma_start(
                xts[0][:], x_h.ap()[0].rearrange("(k p) b t -> p k b t", p=128)
            )

            wqk_sb = const.tile([128, KC, 2 * H], BF16, tag="wqk")
            nc.sync.dma_start(wqk_sb[:], wqk_h.ap().rearrange("(k p) h -> p k h", p=128))
            wv_sb = const.tile([128, KC, H], BF16, tag="wv")
            nc.sync.dma_start(wv_sb[:], wv_h.ap().rearrange("(k p) h -> p k h", p=128))
            bqk_sb = const.tile([128, 1], F32, tag="bqk")
            nc.sync.dma_start(bqk_sb[:], bqk_h.ap())
            bvb_sb = const.tile([128, H], F32, tag="bvb")
            nc.sync.dma_start(bvb_sb[:], bvb_h.ap())

            for pre in range(1, 3):
                xts[pre] = xp.tile(
                    [128, KC, 4, T], BF16, tag="xt", name=f"xt{pre}"
                )
                nc.sync.dma_start(
                    xts[pre][:],
                    x_h.ap()[pre].rearrange("(k p) b t -> p k b t", p=128),
                )

            # v-with-ones-column tiles: persistent ring so the ones column is
            # written once here instead of a per-iteration memset (whose WAR
            # wait on the pool recycle was observed blocking the GpSimd queue
            # for microseconds, stalling the masks behind it).
            vo_ring = []
            for r in range(7):
                vt = const.tile([128, 4, 2, H + 1], BF16, tag=f"vor{r}")
                nc.gpsimd.memset(vt[:, :, :, H:H + 1], 1.0)
                vo_ring.append(vt)

            qts, kts, exs, vos = {}, {}, {}, {}

            # Stage issue order A, B, C with C five quads behind; the kt
            # DMA goes last so its wait on the qt evacuation cannot
            # head-block the prefetch/store DMAs on the Sync queue.
            for i in range(NQ + 5):
                # ---------------- input prefetch ---------------------------
                if 3 <= i + 3 < NQ:
                    xts[i + 3] = xp.tile(
                        [128, KC, 4, T], BF16, tag="xt", name=f"xt{i + 3}"
                    )
                    nc.sync.dma_start(
                        xts[i + 3][:],
                        x_h.ap()[i + 3].rearrange("(k p) b t -> p k b t", p=128),
                    )

                # ---------------- stage A: projections for quad i ----------
                if i < NQ:
                    q = i
                    xt = xts[q]

                    # fused q^T|k^T projection, two 512-col groups of 2
                    # batches, each evacuated separately so its PSUM bank
                    # frees early
                    qt = qkp.tile([128, 4, T], BF16, tag="qt", name=f"qt{q}")
                    for grp, qk_ps in ((0, ps_qk0), (1, ps_qk1)):
                        for k in range(KC):
                            nc.tensor.matmul(
                                qk_ps[:],
                                wqk_sb[:, k, :],
                                xt[:, k, 2 * grp:2 * grp + 2, :],
                                start=(k == 0), stop=(k == KC - 1),
                            )
                        _badd(
                            nc,
                            qt[:, 2 * grp:2 * grp + 2, :].rearrange(
                                "p b t -> p (b t)"
                            ),
                            qk_ps[:],
                            bqk_sb[:],
                        )
                    qts[q] = qt

                    # v in [s, h] layout (x^T chunks stationary)
                    v_ps = ps_v
                    for b in range(4):
                        for si in range(2):
                            for k in range(KC):
                                nc.tensor.matmul(
                                    v_ps[:, 64 * (2 * b + si):64 * (2 * b + si) + 64],
                                    xt[:, k, b, si * 128:(si + 1) * 128],
                                    wv_sb[:, k, :],
                                    start=(k == 0), stop=(k == KC - 1),
                                )
                    vo = vo_ring[q % 7]
                    _badd(
                        nc,
                        vo[:, :, :, 0:H],
                        v_ps[:].rearrange("p (b s h) -> p b s h", b=4, s=2),
                        bvb_sb[:][:, None, None, :],
                    )
                    vos[q] = vo

                # ---------------- stage B: scores/softmax for quad i-1 -----
                if 0 <= i - 1 < NQ:
                    q = i - 1
                    qt, kt = qts[q], kts[q]
                    stA = ps_stA
                    stB = ps_stB
                    for b in range(4):
                        nc.tensor.matmul(
                            stA[:, 256 * b:256 * b + 256], qt[0:H, b, 0:128],
                            kt[:, b, :],
                            start=True, stop=True,
                        )
                        nc.tensor.matmul(
                            stB[:, 128 * b:128 * b + 128], qt[0:H, b, 128:T],
                            kt[:, b, 128:T],
                            start=True, stop=True,
                        )
                    ex = expp.tile([128, 4, 384], BF16, tag="ex", name=f"ex{q}")
                    nc.scalar.activation(
                        ex[:, :, 0:256],
                        stA[:].rearrange("p (b t) -> p b t", b=4),
                        AF.Exp, scale=SCALE,
                    )
                    nc.scalar.activation(
                        ex[:, :, 256:384],
                        stB[:].rearrange("p (b t) -> p b t", b=4),
                        AF.Exp, scale=SCALE,
                    )
                    # causal mask on the two diagonal 128x128 blocks:
                    # keep col >= row, zero the rest (b-independent)
                    nc.gpsimd.affine_select(
                        out=ex[:, :, 0:128], in_=ex[:, :, 0:128],
                        compare_op=mybir.AluOpType.is_ge, fill=0.0,
                        base=0, pattern=[[0, 4], [1, 128]], channel_multiplier=-1,
                    )
                    nc.gpsimd.affine_select(
                        out=ex[:, :, 256:384], in_=ex[:, :, 256:384],
                        compare_op=mybir.AluOpType.is_ge, fill=0.0,
                        base=0, pattern=[[0, 4], [1, 128]], channel_multiplier=-1,
                    )
                    exs[q] = ex
                    del qts[q], kts[q]

                # ---------------- stage C: P@V + normalize + store ---------
                if 0 <= i - 5 < NQ:
                    q = i - 5
                    ex, vo = exs[q], vos[q]
                    # During the pipeline drain (no more projections) the qk
                    # banks are dead; alternate the P@V output onto them so
                    # consecutive drain quads don't serialize on the single
                    # ou bank's WAW.
                    split = i >= NQ and q % 2 == 0
                    for b in range(4):
                        if split:
                            o0 = ps_qk0[:, 128 * b:128 * b + 65]
                            o1 = ps_qk1[:, 128 * b:128 * b + 65]
                        else:
                            o0 = ps_ou[:, 128 * b:128 * b + 65]
                            o1 = ps_ou[:, 512 + 128 * b:512 + 128 * b + 65]
                        nc.tensor.matmul(
                            o0, ex[:, b, 0:128], vo[:, b, 0, :],
                            start=True, stop=True,
                        )
                        nc.tensor.matmul(
                            o1, ex[:, b, 128:256], vo[:, b, 0, :],
                            start=True, stop=False,
                        )
                        nc.tensor.matmul(
                            o1, ex[:, b, 256:384], vo[:, b, 1, :],
                            start=False, stop=True,
                        )
                    rec = op.tile([128, 2, 4, 1], F32, tag="rec", name=f"rec{q}")
                    ot = op.tile([128, 2, 4, H], BF16, tag="ot", name=f"ot{q}")
                    if split:
                        for u, pt in ((0, ps_qk0), (1, ps_qk1)):
                            ouv = pt[:].rearrange("p (b c) -> p b c", b=4)
                            nc.vector.reciprocal(rec[:, u], ouv[:, :, H:H + 1])
                            _bmul(nc, ot[:, u], ouv[:, :, 0:H], rec[:, u])
                    else:
                        ouv = ps_ou[:].rearrange("p (u b c) -> p u b c", u=2, b=4)
                        nc.vector.reciprocal(rec[:], ouv[:, :, :, H:H + 1])
                        _bmul(nc, ot[:], ouv[:, :, :, 0:H], rec[:])
                    nc.sync.dma_start(
                        out_h.ap()[q].rearrange("u p b h -> p u b h"), ot[:]
                    )
                    del exs[q], vos[q]
                    # During the drain the PE has little work and HAM drops
                    # it to half clock, doubling the remaining matmul
                    # latency. Keep it warm with dummy matmuls into the dead
                    # v-projection bank between drain C stages (skip after
                    # the last one so the teardown isn't delayed).
                    if NQ <= i < NQ + 4:
                        for _ in range(6):
                            nc.tensor.matmul(
                                ps_v[:], wsb[:, 0:128], wsb[:],
                                start=True, stop=True,
                            )

                # k^T half of quad i re-based to partition 0 (only DMA can
                # shift partitions) so the score matmul operands share a
                # base. Issued last so its wait on the qt evacuation can't
                # head-block the store/prefetch DMAs on the Sync queue.
                if i < NQ:
                    q = i
                    kt = qkp.tile([H, 4, T], BF16, tag="kt", name=f"kt{q}")
                    nc.sync.dma_start(kt[:], qts[q][64:128])
                    kts[q] = kt

            for ctx in reversed(ctxs):
                ctx.__exit__(None, None, None)
    nc.compile()
    return nc


_NC_CACHE = None


def _get_nc():
    global _NC_CACHE
    if _NC_CACHE is None:
        _NC_CACHE = build_nc()
    return _NC_CACHE


def prep_in_maps(inp, Wv, bv, Wk, bk, Wq, bq):
    """Host-side shard + layout prep. Returns the 8 per-core input maps."""
    bf16 = ml_dtypes.bfloat16
    wqk_b = np.ascontiguousarray(
        np.concatenate(
            [np.asarray(Wq, np.float32), np.asarray(Wk, np.float32)], axis=1
        ).astype(bf16)
    )
    wv_b = np.ascontiguousarray(np.asarray(Wv, np.float32).astype(bf16))
    bqk_c = np.ascontiguousarray(
        np.concatenate(
            [np.asarray(bq, np.float32).reshape(H), np.asarray(bk, np.float32).reshape(H)]
        ).reshape(128, 1)
    )
    bvb = np.ascontiguousarray(
        np.tile(np.asarray(bv, np.float32).reshape(1, H), (128, 1))
    )
    inp = np.asarray(inp, np.float32)
    in_maps = []
    for c in range(N_CORES):
        shard = inp[c * NB:(c + 1) * NB]                    # [NB, T, C]
        x_t = np.ascontiguousarray(
            shard.reshape(NQ, 4, T, C).transpose(0, 3, 1, 2).astype(bf16)
        )                                                    # [NQ, C, 4, T]
        in_maps.append({
            "x": x_t, "wqk": wqk_b, "wv": wv_b, "bqk": bqk_c, "bvb": bvb,
        })
    return in_maps


def unpack_out(results):
    """results: list of per-core dicts -> full [B, T, H] float32 output."""
    outs = []
    for c in range(N_CORES):
        o = np.asarray(results[c]["out"], dtype=np.float32)  # [NQ, 2, 128, 4, H]
        outs.append(o.transpose(0, 3, 1, 2, 4).reshape(NB, T, H))
    return np.concatenate(outs, axis=0)


def kernel(inp, Wv, bv, Wk, bk, Wq, bq):
    in_maps = prep_in_maps(inp, Wv, bv, Wk, bk, Wq, bq)
    nc = _get_nc()
    res = run_bass_kernel_spmd(nc, in_maps, core_ids=list(range(N_CORES)))
    return unpack_out(res.results)


# revision 57
# speedup vs baseline: 1.0263x; 1.0263x over previous
"""Trainium2 Bass kernel for a causal single-head attention layer.

reference:
    v = inp @ Wv + bv; k = inp @ Wk + bk; q = inp @ Wq + bq      # [B,T,H]
    W = softmax(causal_mask(k @ q^T / sqrt(C)))                  # [B,T,T]
    out = W @ v                                                  # [B,T,H]

B=512, T=256, C=384, H=64. Pure data parallel over 8 NeuronCores
(64 batches each); batches are processed in QUADS (4 at a time, 16
iterations per core) with a deep software pipeline:

    iteration i issues:  A = projections(quad i)       + kt DMA
                         B = scores+exp+mask(quad i-1)
                         C = P@V+normalize+store(quad i-5)

The deep C offset gives the scores->exp->mask cross-engine chain
(~4us of serial latency) four iterations of slack, so the in-order
Tensor queue never waits on it; measured steady-state is one quad per
~3.5us with a single unbroken full-clock (HAM K=8/8) region.

Scheduling notes (all learned from NTFF traces):
  * All 8 PSUM banks are persistent per-role tiles. The qk projection
    uses two single-bank tiles evacuated separately so each bank frees
    ~1.5us into its iteration (a combined evacuation made the next
    quad's projections the critical path).
  * Input x^T tiles are prefetched 3 iterations ahead on the Sync
    queue (786KB transfers take ~2.2us on SDMA; shorter distances
    caused multi-us PE stalls and HAM down-clocks).
  * The causal mask (needed only on the two diagonal 128x128 blocks of
    the packed score layout) is applied by gpsimd affine_select after
    exp; max-subtraction is skipped (|scores/sqrt(C)| < ~3, exp cannot
    overflow; softmax is shift-invariant).
  * Softmax normalization rides a ones-column appended to V (written
    once into a persistent ring of vo tiles) so the P@V matmuls also
    emit row sums; one reciprocal + one broadcast multiply per t-block
    finish it.
  * During the pipeline drain the dead qk banks double-buffer the P@V
    output so the tail quads don't serialize on the single ou bank.
  * Output is stored as bf16 and cast to f32 on the host (within the
    accuracy budget; measured rel err ~4e-3).

Layout: scores are computed transposed (S^T[s,t], s on partitions) with
lhsT=q^T slices / rhs=k^T (k^T re-based to partition 0 by an SBUF DMA);
exp(S^T) lands in one [128, 4, 384] bf16 tile per quad which directly
provides the stationary chunks for the P@V matmuls. V is projected
directly in [s,h] layout (x^T chunks stationary), so no transposes
anywhere; the host pre-transposes inp to [NQ, C, 4, T].
"""

import numpy as np
import ml_dtypes

import concourse.bass as bass
import concourse.bacc as bacc
import concourse.mybir as mybir
import concourse.tile as tile
from concourse.bass import broadcast_tensor_aps
from concourse.bass_utils import run_bass_kernel_spmd

N_CORES = 8
B, T, C, H = 512, 256, 384, 64
NB = B // N_CORES          # batches per core
NQ = NB // 4               # batch quads per core
KC = C // 128              # contraction chunks
SCALE = C ** (-0.5)
F32 = mybir.dt.float32
BF16 = mybir.dt.bfloat16
AF = mybir.ActivationFunctionType


def _bmul(nc, out, a, b):
    a2, b2 = broadcast_tensor_aps(a, b)
    nc.vector.tensor_tensor(out, a2, b2, op=mybir.AluOpType.mult)


def _badd(nc, out, a, b):
    a2, b2 = broadcast_tensor_aps(a, b)
    nc.vector.tensor_tensor(out, a2, b2, op=mybir.AluOpType.add)


def build_nc():
    nc = bacc.Bacc("TRN2", target_bir_lowering=False, debug=False)
    x_h = nc.declare_dram_parameter("x", [NQ, C, 4, T], BF16, isOutput=False)
    wqk_h = nc.declare_dram_parameter("wqk", [C, 2 * H], BF16, isOutput=False)
    wv_h = nc.declare_dram_parameter("wv", [C, H], BF16, isOutput=False)
    bqk_h = nc.declare_dram_parameter("bqk", [128, 1], F32, isOutput=False)
    bvb_h = nc.declare_dram_parameter("bvb", [128, H], F32, isOutput=False)
    # out[q, u, p, b, h] = attention output for batch 4q+b, t = u*128+p
    out_h = nc.declare_dram_parameter("out", [NQ, 2, 128, 4, H], BF16, isOutput=True)

    with tile.TileContext(nc) as tc:
        with (
            tc.tile_pool(name="const", bufs=1) as const,
            tc.tile_pool(name="xp", bufs=5) as xp,
            tc.tile_pool(name="qkp", bufs=3) as qkp,
            tc.tile_pool(name="exp", bufs=6) as expp,
            tc.tile_pool(name="op", bufs=3) as op,
        ):
            # PE warm-up: dummy matmuls with no input dependencies so the HAM
            # clock gate ramps while the first input DMA streams.
            with tc.tile_pool(name="warm_ps", bufs=1, space="PSUM") as warm_ps:
                wsb = const.tile([128, 512], BF16, tag="wsb")
                nc.gpsimd.memset(wsb[:], 1.0)
                wps = warm_ps.tile([128, 512], F32, tag="wps")
                for _ in range(8):
                    nc.tensor.matmul(
                        wps[:], wsb[:, 0:128], wsb[:], start=True, stop=True
                    )

            ctxs = []

            def psum_pool(name, bufs=1):
                ctx = tc.tile_pool(name=name, bufs=bufs, space="PSUM")
                ctxs.append(ctx)
                return ctx.__enter__()

            # All 8 PSUM banks as persistent per-role tiles. The qk
            # projection uses two single-bank tiles (one per 2-batch group)
            # whose evacuations are split likewise, so each bank frees
            # ~1.5us into its iteration and the next quad's projections
            # never wait on a late evacuation.
            ps_ring = psum_pool("ps_ring")
            ps_qk0 = ps_ring.tile([128, 512], F32, tag="qk0", name="qk0")
            ps_qk1 = ps_ring.tile([128, 512], F32, tag="qk1", name="qk1")
            ps_v = ps_ring.tile([128, 512], F32, tag="psv", name="psv")
            ps_stA = ps_ring.tile([128, 1024], F32, tag="stA", name="stA")
            ps_stB = ps_ring.tile([128, 512], F32, tag="stB", name="stB")
            ps_ou = ps_ring.tile([128, 1024], F32, tag="psou", name="psou")

            # first quad's input DMA goes ahead of the constant loads so the
            # projection matmuls can start as soon as the warm-up drains.
            xts = {}
            xts[0] = xp.tile([128, KC, 4, T], BF16, tag="xt", name="xt0")
            nc.sync.dma_start(
                xts[0][:], x_h.ap()[0].rearrange("(k p) b t -> p k b t", p=128)
            )

            wqk_sb = const.tile([128, KC, 2 * H], BF16, tag="wqk")
            nc.sync.dma_start(wqk_sb[:], wqk_h.ap().rearrange("(k p) h -> p k h", p=128))
            wv_sb = const.tile([128, KC, H], BF16, tag="wv")
            nc.sync.dma_start(wv_sb[:], wv_h.ap().rearrange("(k p) h -> p k h", p=128))
            bqk_sb = const.tile([128, 1], F32, tag="bqk")
            nc.sync.dma_start(bqk_sb[:], bqk_h.ap())
            bvb_sb = const.tile([128, H], F32, tag="bvb")
            nc.sync.dma_start(bvb_sb[:], bvb_h.ap())

            for pre in range(1, 3):
                xts[pre] = xp.tile(
                    [128, KC, 4, T], BF16, tag="xt", name=f"xt{pre}"
                )
                nc.sync.dma_start(
                    xts[pre][:],
                    x_h.ap()[pre].rearrange("(k p) b t -> p k b t", p=128),
                )

            # v-with-ones-column tiles: persistent ring so the ones column is
            # written once here instead of a per-iteration memset (whose WAR
            # wait on the pool recycle was observed blocking the GpSimd queue
            # for microseconds, stalling the masks behind it).
            vo_ring = []
            for r in range(7):
                vt = const.tile([128, 4, 2, H + 1], BF16, tag=f"vor{r}")
                nc.gpsimd.memset(vt[:, :, :, H:H + 1], 1.0)
                vo_ring.append(vt)

            qts, kts, exs, vos = {}, {}, {}, {}

            # Stage issue order A, B, C with C five quads behind; the kt
            # DMA goes last so its wait on the qt evacuation cannot
            # head-block the prefetch/store DMAs on the Sync queue.
            for i in range(NQ + 5):
                # ---------------- input prefetch ---------------------------
                if 3 <= i + 3 < NQ:
                    xts[i + 3] = xp.tile(
                        [128, KC, 4, T], BF16, tag="xt", name=f"xt{i + 3}"
                    )
                    nc.sync.dma_start(
                        xts[i + 3][:],
                        x_h.ap()[i + 3].rearrange("(k p) b t -> p k b t", p=128),
                    )

                # ---------------- stage A: projections for quad i ----------
                if i < NQ:
                    q = i
                    xt = xts[q]

                    # fused q^T|k^T projection, two 512-col groups of 2
                    # batches, each evacuated separately so its PSUM bank
                    # frees early
                    qt = qkp.tile([128, 4, T], BF16, tag="qt", name=f"qt{q}")
                    for grp, qk_ps in ((0, ps_qk0), (1, ps_qk1)):
                        for k in range(KC):
                            nc.tensor.matmul(
                                qk_ps[:],
                                wqk_sb[:, k, :],
                                xt[:, k, 2 * grp:2 * grp + 2, :],
                                start=(k == 0), stop=(k == KC - 1),
                            )
                        _badd(
                            nc,
                            qt[:, 2 * grp:2 * grp + 2, :].rearrange(
                                "p b t -> p (b t)"
                            ),
                            qk_ps[:],
                            bqk_sb[:],
                        )
                    qts[q] = qt

                    # v in [s, h] layout (x^T chunks stationary)
                    v_ps = ps_v
                    for b in range(4):
                        for si in range(2):
                            for k in range(KC):
                                nc.tensor.matmul(
                                    v_ps[:, 64 * (2 * b + si):64 * (2 * b + si) + 64],
                                    xt[:, k, b, si * 128:(si + 1) * 128],
                                    wv_sb[:, k, :],
                                    start=(k == 0), stop=(k == KC - 1),
                                )
                    vo = vo_ring[q % 7]
                    _badd(
                        nc,
                        vo[:, :, :, 0:H],
                        v_ps[:].rearrange("p (b s h) -> p b s h", b=4, s=2),
                        bvb_sb[:][:, None, None, :],
                    )
                    vos[q] = vo

                # ---------------- stage B: scores/softmax for quad i-1 -----
                if 0 <= i - 1 < NQ:
                    q = i - 1
                    qt, kt = qts[q], kts[q]
                    stA = ps_stA
                    stB = ps_stB
                    for b in range(4):
                        nc.tensor.matmul(
                            stA[:, 256 * b:256 * b + 256], qt[0:H, b, 0:128],
                            kt[:, b, :],
                            start=True, stop=True,
                        )
                        nc.tensor.matmul(
                            stB[:, 128 * b:128 * b + 128], qt[0:H, b, 128:T],
                            kt[:, b, 128:T],
                            start=True, stop=True,
                        )
                    ex = expp.tile([128, 4, 384], BF16, tag="ex", name=f"ex{q}")
                    nc.scalar.activation(
                        ex[:, :, 0:256],
                        stA[:].rearrange("p (b t) -> p b t", b=4),
                        AF.Exp, scale=SCALE,
                    )
                    nc.scalar.activation(
                        ex[:, :, 256:384],
                        stB[:].rearrange("p (b t) -> p b t", b=4),
                        AF.Exp, scale=SCALE,
                    )
                    # causal mask on the two diagonal 128x128 blocks:
                    # keep col >= row, zero the rest (b-independent)
                    nc.gpsimd.affine_select(
                        out=ex[:, :, 0:128], in_=ex[:, :, 0:128],
                        compare_op=mybir.AluOpType.is_ge, fill=0.0,
                        base=0, pattern=[[0, 4], [1, 128]], channel_multiplier=-1,
                    )
                    nc.gpsimd.affine_select(
                        out=ex[:, :, 256:384], in_=ex[:, :, 256:384],
                        compare_op=mybir.AluOpType.is_ge, fill=0.0,
                        base=0, pattern=[[0, 4], [1, 128]], channel_multiplier=-1,
                    )
                    exs[q] = ex
                    del qts[q], kts[q]

                # ---------------- stage C: P@V + normalize + store ---------
                if 0 <= i - 5 < NQ:
                    q = i - 5
                    ex, vo = exs[q], vos[q]
                    # During the pipeline drain (no more projections) the qk
                    # banks are dead; alternate the P@V output onto them so
                    # consecutive drain quads don't serialize on the single
                    # ou bank's WAW.
                    split = i >= NQ and q % 2 == 0
                    for b in range(4):
                        if split:
                            o0 = ps_qk0[:, 128 * b:128 * b + 65]
                            o1 = ps_qk1[:, 128 * b:128 * b + 65]
                        else:
                            o0 = ps_ou[:, 128 * b:128 * b + 65]
                            o1 = ps_ou[:, 512 + 128 * b:512 + 128 * b + 65]
                        nc.tensor.matmul(
                            o0, ex[:, b, 0:128], vo[:, b, 0, :],
                            start=True, stop=True,
                        )
                        nc.tensor.matmul(
                            o1, ex[:, b, 128:256], vo[:, b, 0, :],
                            start=True, stop=False,
                        )
                        nc.tensor.matmul(
                            o1, ex[:, b, 256:384], vo[:, b, 1, :],
                            start=False, stop=True,
                        )
                    rec = op.tile([128, 2, 4, 1], F32, tag="rec", name=f"rec{q}")
                    ot = op.tile([128, 2, 4, H], BF16, tag="ot", name=f"ot{q}")
                    if split:
                        for u, pt in ((0, ps_qk0), (1, ps_qk1)):
                            ouv = pt[:].rearrange("p (b c) -> p b c", b=4)
                            nc.vector.reciprocal(rec[:, u], ouv[:, :, H:H + 1])
                            _bmul(nc, ot[:, u], ouv[:, :, 0:H], rec[:, u])
                    else:
                        ouv = ps_ou[:].rearrange("p (u b c) -> p u b c", u=2, b=4)
                        nc.vector.reciprocal(rec[:], ouv[:, :, :, H:H + 1])
                        _bmul(nc, ot[:], ouv[:, :, :, 0:H], rec[:])
                    nc.sync.dma_start(
                        out_h.ap()[q].rearrange("u p b h -> p u b h"), ot[:]
                    )
                    del exs[q], vos[q]

                # k^T half of quad i re-based to partition 0 (only DMA can
                # shift partitions) so the score matmul operands share a
                # base. Issued last so its wait on the qt evacuation can't
                # head-block the store/prefetch DMAs on the Sync queue.
                if i < NQ:
                    q = i
                    kt = qkp.tile([H, 4, T], BF16, tag="kt", name=f"kt{q}")
                    nc.sync.dma_start(kt[:], qts[q][64:128])
                    kts[q] = kt

            for ctx in reversed(ctxs):
                ctx.__exit__(None, None, None)
    nc.compile()
    return nc


_NC_CACHE = None


def _get_nc():
    global _NC_CACHE
    if _NC_CACHE is None:
        _NC_CACHE = build_nc()
    return _NC_CACHE


def prep_in_maps(inp, Wv, bv, Wk, bk, Wq, bq):
    """Host-side shard + layout prep. Returns the 8 per-core input maps."""
    bf16 = ml_dtypes.bfloat16
    wqk_b = np.ascontiguousarray(
        np.concatenate(
            [np.asarray(Wq, np.float32), np.asarray(Wk, np.float32)], axis=1
        ).astype(bf16)
    )
    wv_b = np.ascontiguousarray(np.asarray(Wv, np.float32).astype(bf16))
    bqk_c = np.ascontiguousarray(
        np.concatenate(
            [np.asarray(bq, np.float32).reshape(H), np.asarray(bk, np.float32).reshape(H)]
        ).reshape(128, 1)
    )
    bvb = np.ascontiguousarray(
        np.tile(np.asarray(bv, np.float32).reshape(1, H), (128, 1))
    )
    inp = np.asarray(inp, np.float32)
    in_maps = []
    for c in range(N_CORES):
        shard = inp[c * NB:(c + 1) * NB]                    # [NB, T, C]
        x_t = np.ascontiguousarray(
            shard.reshape(NQ, 4, T, C).transpose(0, 3, 1, 2).astype(bf16)
        )                                                    # [NQ, C, 4, T]
        in_maps.append({
            "x": x_t, "wqk": wqk_b, "wv": wv_b, "bqk": bqk_c, "bvb": bvb,
        })
    return in_maps


def unpack_out(results):
    """results: list of per-core dicts -> full [B, T, H] float32 output."""
    outs = []
    for c in range(N_CORES):
        o = np.asarray(results[c]["out"], dtype=np.float32)  # [NQ, 2, 128, 4, H]
        outs.append(o.transpose(0, 3, 1, 2, 4).reshape(NB, T, H))
    return np.concatenate(outs, axis=0)


def kernel(inp, Wv, bv, Wk, bk, Wq, bq):
    in_maps = prep_in_maps(inp, Wv, bv, Wk, bk, Wq, bq)
    nc = _get_nc()
    res = run_bass_kernel_spmd(nc, in_maps, core_ids=list(range(N_CORES)))
    return unpack_out(res.results)
